# revision 1
# baseline (speedup 1.0000x reference)
"""Trainium2 Bass kernel for GQA attention with QK-RMSNorm, partial mRoPE,
causal mask and sigmoid output gate (nn_Attention_70557722739259).

Model dims: B=2, T=2048, D=2048, N=16 Q heads, K=2 KV heads, H=256.
Sharding over 8 NeuronCores: DP=2 over batch x TP=4 over head groups
(4 Q heads + their shared KV head per core). Each core computes a partial
output projection over its 4 heads; the host sums the 4 partials per batch
(Megatron-style unshard).

Per-core device algorithm (all matmuls bf16, softmax f32):
  phase 1 (two half-T passes): projections q/gate/k/v from host-pre-transposed
           x^T, RMSNorm stats via DVE tensor_tensor_reduce, sigmoid gate,
           batched Sqrt for rms, RoPE + norm scale token-major, PE-transpose
           Q,K to feature-major (H, T).
  phase 2: per head, S^T = K^T.T @ Q^T tiles (keys on partitions), exp via
           ACT (1/16 scale folded in; no max subtraction - scores are O(10)),
           multiplicative 0/1 causal mask on diagonal blocks, AV + softmax
           denominator accumulated in PSUM via ones-column in V.
  phase 3: o-proj partial from gated qkv^T (feature-major via PE transpose).
"""
import sys
sys.path.insert(0, "/opt/trn_rl_repo")
import numpy as np
import ml_dtypes

from concourse import bacc, tile, mybir
from concourse import bass_utils
from concourse.masks import make_identity

BF16 = ml_dtypes.bfloat16
F32 = mybir.dt.float32
BF = mybir.dt.bfloat16

B, T, D = 2, 2048, 2048
N_HEADS, N_KV, H = 16, 2, 256
HEADS_PC = 4            # q heads per core (TP=4)
ROPE_THETA = 1000000
ROTARY = 64             # int(H * 0.25)
FREQ = 32
NORM_EPS = 1e-6
K_MASK = -2.3819763e38
SCALE = H ** (-0.5)     # 1/16

TT = T // 128           # 16 token tiles
DC = D // 128           # 16 contraction chunks

LAST_RESULT = None
LAST_IN_MAPS = None
_COMPILED = {}


def _build(mode="causal", apply_w=False, phases=(1, 2, 3), n_halves=2, n_heads_dbg=HEADS_PC):
    nc = bacc.Bacc("TRN2", target_bir_lowering=False, debug=False,
                   enable_asserts=True, num_devices=8)
    Act = mybir.ActivationFunctionType
    Alu = mybir.AluOpType

    xT = nc.dram_tensor("xT", (D, T), BF, kind="ExternalInput").ap()
    wq = nc.dram_tensor("wq", (D, HEADS_PC * 2 * H), BF, kind="ExternalInput").ap()
    wkv = nc.dram_tensor("wkv", (D, 2 * H), BF, kind="ExternalInput").ap()
    wo = nc.dram_tensor("wo", (HEADS_PC * H, D), BF, kind="ExternalInput").ap()
    cc = nc.dram_tensor("cc", (T, ROTARY), BF, kind="ExternalInput").ap()
    ss = nc.dram_tensor("ss", (T, ROTARY), BF, kind="ExternalInput").ap()
    if apply_w:
        qw = nc.dram_tensor("qw", (128, H), F32, kind="ExternalInput").ap()
        kw = nc.dram_tensor("kw", (128, H), F32, kind="ExternalInput").ap()
    if mode == "arbitrary":
        am = nc.dram_tensor("am", (T, T), F32, kind="ExternalInput").ap()
    out = nc.dram_tensor("out", (T, D), BF, kind="ExternalOutput").ap()

    with tile.TileContext(nc) as tc:
        with tc.tile_pool(name="const", bufs=1) as constp, \
             tc.tile_pool(name="attn", bufs=1) as ap_, \
             tc.tile_pool(name="psum", bufs=1, space="PSUM") as psum:

            # ---- constants ----
            ident = constp.tile([128, 128], BF, tag="ident", name="ident")
            make_identity(nc, ident[:])
            epst = constp.tile([128, 1], F32, tag="epst", name="epst")
            nc.gpsimd.memset(epst[:], NORM_EPS)
            tri = []
            if mode == "causal":
                for r in range(4):
                    t = constp.tile([128, 512], BF, tag=f"tri{r}", name=f"tri{r}")
                    nc.gpsimd.memset(t[:], 1.0)
                    # keep (1.0) where col - part - 128*r >= 0, else fill 0.0
                    nc.gpsimd.affine_select(
                        out=t[:], in_=t[:], compare_op=Alu.is_ge, fill=0.0,
                        base=-128 * r, channel_multiplier=-1, pattern=[[1, 512]])
                    tri.append(t)
            if apply_w:
                qw_sb = constp.tile([128, H], F32, tag="qw", name="qw")
                kw_sb = constp.tile([128, H], F32, tag="kw", name="kw")
                nc.sync.dma_start(qw_sb[:], qw[:])
                nc.sync.dma_start(kw_sb[:], kw[:])

            # ---- persistent attention tensors (span phase 1 -> 2) ----
            QT = [[ap_.tile([128, T], BF, tag=f"QT{h}_{c}", name=f"QT{h}_{c}")
                   for c in range(2)] for h in range(HEADS_PC)]
            KT = [ap_.tile([128, T], BF, tag=f"KT{c}", name=f"KT{c}")
                  for c in range(2)]
            V = [ap_.tile([128, H + 1], BF, tag=f"V{i}", name=f"V{i}")
                 for i in range(TT)]
            gate = [[ap_.tile([128, H], BF, tag=f"g{h}_{i}", name=f"g{h}_{i}")
                     for i in range(TT)] for h in range(HEADS_PC)]

            # ================= phase 1 (two half-T passes) =================
            with tc.tile_pool(name="praw", bufs=1) as praw, \
                 tc.tile_pool(name="proj", bufs=1) as proj, \
                 tc.tile_pool(name="p1w", bufs=3) as p1w, \
                 tc.tile_pool(name="p1c", bufs=3) as p1c:
                wkv_sb = [proj.tile([128, 2 * H], BF, tag=f"wkv{d}",
                                    name=f"wkv{d}") for d in range(DC)]
                for d in range(DC):
                    nc.sync.dma_start(wkv_sb[d][:],
                                      wkv[d * 128:(d + 1) * 128, :])

                n_half = HEADS_PC * 8 + 8   # ssq columns per half (q then k)

                for half in range(n_halves):
                    t0 = half * 1024
                    xT_sb = []
                    for d in range(DC):
                        xt_ = proj.tile([128, 1024], BF, tag=f"xT{d}",
                                        name=f"xT{d}_{half}")
                        nc.sync.dma_start(
                            xt_[:], xT[d * 128:(d + 1) * 128, t0:t0 + 1024])
                        xT_sb.append(xt_)
                    cc_sb, ss_sb = [], []
                    for tl in range(8):
                        ti = half * 8 + tl
                        c_ = praw.tile([128, ROTARY], BF, tag=f"cc{tl}",
                                       name=f"cc{tl}_{half}")
                        s_ = praw.tile([128, ROTARY], BF, tag=f"ss{tl}",
                                       name=f"ss{tl}_{half}")
                        nc.sync.dma_start(c_[:], cc[ti * 128:(ti + 1) * 128, :])
                        nc.sync.dma_start(s_[:], ss[ti * 128:(ti + 1) * 128, :])
                        cc_sb.append(c_)
                        ss_sb.append(s_)

                    ssqall = praw.tile([128, n_half], F32, tag="ssq", bufs=2,
                                       name=f"ssq{half}")
                    q_raw = [[praw.tile([128, H], BF, tag=f"qr{h}_{tl}",
                                        name=f"qr{h}_{tl}_{half}")
                              for tl in range(8)] for h in range(HEADS_PC)]
                    k_raw = [praw.tile([128, H], BF, tag=f"kr{tl}",
                                       name=f"kr{tl}_{half}")
                             for tl in range(8)]

                    # ---- pass A: projections ----
                    for tl in range(8):
                        ti = half * 8 + tl
                        pk = psum.tile([128, 2 * H], F32, tag="mm", bufs=2,
                                       name=f"pk{ti}")
                        for d in range(DC):
                            nc.tensor.matmul(
                                pk[:], xT_sb[d][:, tl * 128:(tl + 1) * 128],
                                wkv_sb[d][:], start=(d == 0), stop=(d == DC - 1))
                        kvt = p1w.tile([128, 2 * H], BF, tag="fulltmp",
                                       name=f"kvt{ti}")
                        nc.scalar.copy(kvt[:], pk[:])
                        nc.vector.tensor_copy(k_raw[tl][:], kvt[:, 0:H])
                        nc.vector.tensor_copy(V[ti][:, 0:H], kvt[:, H:2 * H])
                        nc.gpsimd.memset(V[ti][:, H:H + 1], 1.0)
                        junk = p1w.tile([128, H], F32, tag="junk", name=f"jk{ti}")
                        nc.scalar.square(junk[:], k_raw[tl][:])
                        nc.vector.reduce_sum(ssqall[:, 32 + tl:32 + tl + 1],
                                             junk[:], axis=mybir.AxisListType.X)

                    for h in range(n_heads_dbg):
                        wq_h = []
                        for d in range(DC):
                            w_ = proj.tile([128, 512], BF, tag=f"wq{d}",
                                           name=f"wq{d}_{half}_{h}")
                            nc.sync.dma_start(
                                w_[:], wq[d * 128:(d + 1) * 128,
                                          h * 512:(h + 1) * 512])
                            wq_h.append(w_)
                        for tl in range(8):
                            ti = half * 8 + tl
                            pq = psum.tile([128, 2 * H], F32, tag="mm", bufs=2,
                                           name=f"pq{h}_{ti}")
                            for d in range(DC):
                                nc.tensor.matmul(
                                    pq[:], xT_sb[d][:, tl * 128:(tl + 1) * 128],
                                    wq_h[d][:], start=(d == 0),
                                    stop=(d == DC - 1))
                            qgt = p1w.tile([128, 2 * H], BF, tag="fulltmp",
                                           name=f"qgt{h}_{ti}")
                            nc.scalar.copy(qgt[:], pq[:])
                            nc.vector.tensor_copy(q_raw[h][tl][:], qgt[:, 0:H])
                            nc.scalar.activation(gate[h][ti][:], qgt[:, H:2 * H],
                                                 Act.Sigmoid)
                            junk = p1w.tile([128, H], F32, tag="junk",
                                            name=f"jq{h}_{ti}")
                            nc.scalar.square(junk[:], q_raw[h][tl][:])
                            nc.vector.reduce_sum(
                                ssqall[:, h * 8 + tl:h * 8 + tl + 1],
                                junk[:], axis=mybir.AxisListType.X)

                    # ---- pass B: batched rms scales for this half ----
                    rms = praw.tile([128, n_half], F32, tag="rms", bufs=2,
                                    name=f"rms{half}")
                    rinv = praw.tile([128, n_half], F32, tag="rinv", bufs=2,
                                     name=f"rinv{half}")
                    nc.scalar.activation(rms[:], ssqall[:], Act.Sqrt,
                                         scale=1.0 / H, bias=epst[:])
                    nc.vector.reciprocal(rinv[:], rms[:])

                    # ---- pass C: rope + norm scale + transpose ----
                    def rope_pass(raw, col, dst_tiles, tl, w_sb, nm):
                        ti = half * 8 + tl
                        src = raw
                        if apply_w:
                            srw = p1c.tile([128, H], F32, tag="srw",
                                           name=f"srw{nm}")
                            nc.vector.tensor_mul(srw[:], raw[:], w_sb[:])
                            src = srw
                        rot = p1c.tile([128, ROTARY], BF, tag="rot",
                                       name=f"rot{nm}")
                        t2 = p1c.tile([128, ROTARY], BF, tag="rot2",
                                      name=f"rot2{nm}")
                        nc.vector.tensor_mul(rot[:], src[:, 0:ROTARY],
                                             cc_sb[tl][:])
                        nc.vector.tensor_mul(t2[:, 0:FREQ], src[:, FREQ:ROTARY],
                                             ss_sb[tl][:, 0:FREQ])
                        nc.vector.tensor_mul(t2[:, FREQ:ROTARY], src[:, 0:FREQ],
                                             ss_sb[tl][:, FREQ:ROTARY])
                        nc.vector.tensor_add(rot[:], rot[:], t2[:])
                        tok = p1c.tile([128, H], BF, tag="tok", name=f"tok{nm}")
                        rv = rinv[:, col:col + 1]
                        nc.vector.tensor_scalar_mul(tok[:, 0:ROTARY], rot[:], rv)
                        nc.vector.tensor_scalar_mul(tok[:, ROTARY:H],
                                                    src[:, ROTARY:H], rv)
                        for c2 in range(2):
                            tp = psum.tile([128, 128], BF, tag="tp", bufs=2,
                                           name=f"tp{nm}_{c2}")
                            nc.tensor.transpose(
                                tp[:], tok[:, c2 * 128:(c2 + 1) * 128], ident[:])
                            nc.any.tensor_copy(
                                dst_tiles[c2][:, ti * 128:(ti + 1) * 128], tp[:])

                    for tl in range(8):
                        rope_pass(k_raw[tl], 32 + tl, KT, tl,
                                  kw_sb if apply_w else None, f"k{half}_{tl}")
                    for h in range(n_heads_dbg):
                        for tl in range(8):
                            rope_pass(q_raw[h][tl], h * 8 + tl, QT[h], tl,
                                      qw_sb if apply_w else None,
                                      f"q{h}_{half}_{tl}")

            # ================= phases 2+3 =================
            if phases == (1,):
                # debug: dump K^T and QT[0] so HW output is defined
                with tc.tile_pool(name="dbg", bufs=2) as dbg:
                    for idx, tile_ in enumerate([KT[0], KT[1], QT[0][0],
                                                 QT[0][1], QT[1][0], QT[1][1]]):
                        dt_ = dbg.tile([128, T], BF, tag="dbg", name=f"dbg{idx}")
                        nc.vector.tensor_copy(dt_[:], tile_[:])
                        nc.sync.dma_start(out[idx * 128:(idx + 1) * 128, :], dt_[:])
                    for idx2 in range(6, 16):
                        ti2 = idx2 - 6
                        dt_ = dbg.tile([128, T], BF, tag="dbg", name=f"dbg{idx2}")
                        nc.gpsimd.memset(dt_[:], 0.0)
                        nc.vector.tensor_copy(dt_[:, 0:H], V[ti2][:, 0:H])
                        nc.vector.tensor_copy(dt_[:, H:2 * H], gate[0][ti2][:])
                        nc.sync.dma_start(out[idx2 * 128:(idx2 + 1) * 128, :],
                                          dt_[:])
            if phases != (1,):
              with tc.tile_pool(name="p23", bufs=1) as p23, \
                 tc.tile_pool(name="p2w", bufs=3) as p2w:
                  qkvgT = [p23.tile([128, T], BF, tag=f"qkT{c}", name=f"qkT{c}")
                           for c in range(2 * HEADS_PC)]
                  wo_sb = [p23.tile([128, D], BF, tag=f"wo{c}", name=f"wo{c}")
                           for c in range(2 * HEADS_PC)]
                  for c in range(2 * HEADS_PC):
                      nc.sync.dma_start(wo_sb[c][:], wo[c * 128:(c + 1) * 128, :])

                  for j in range(4):              # q blocks of 512
                    kmax = 4 * (j + 1) if mode == "causal" else TT
                    for h in range(HEADS_PC):
                          av = [psum.tile([128, H + 1], F32, tag=f"av{s}", bufs=1,
                                          name=f"av{h}_{j}_{s}") for s in range(4)]
                          for i in range(kmax):   # key chunks of 128
                              st = psum.tile([128, 512], F32, tag="mm", bufs=2,
                                             name=f"st{h}_{j}_{i}")
                              for c2 in range(2):
                                  nc.tensor.matmul(
                                      st[:],
                                      KT[c2][:, i * 128:(i + 1) * 128],
                                      QT[h][c2][:, j * 512:(j + 1) * 512],
                                      start=(c2 == 0), stop=(c2 == 1))
                              if mode == "arbitrary":
                                  amt = p2w.tile([128, 512], F32, tag="amt",
                                                 name=f"am{h}_{j}_{i}")
                                  nc.sync.dma_start(
                                      amt[:], am[i * 128:(i + 1) * 128,
                                                 j * 512:(j + 1) * 512])
                                  nc.vector.tensor_add(st[:], st[:], amt[:])
                              pT = p2w.tile([128, 512], BF, tag="pT",
                                            name=f"pT{h}_{j}_{i}")
                              nc.scalar.activation(pT[:], st[:], Act.Exp,
                                                   scale=SCALE)
                              if mode == "causal" and i >= 4 * j:
                                  nc.vector.tensor_mul(pT[:], pT[:],
                                                       tri[i - 4 * j][:])
                              for s in range(4):
                                  last_i = (4 * j + s) if mode == "causal" \
                                      else (kmax - 1)
                                  if i > last_i:
                                      continue
                                  nc.tensor.matmul(
                                      av[s][:], pT[:, s * 128:(s + 1) * 128],
                                      V[i][:], start=(i == 0),
                                      stop=(i == last_i))
                          for s in range(4):
                              ti = 4 * j + s
                              avs = p2w.tile([128, H + 1], F32, tag="avs",
                                             name=f"avs{h}_{ti}")
                              nc.vector.tensor_copy(avs[:], av[s][:])
                              rec = p2w.tile([128, 1], F32, tag="rec",
                                             name=f"rec{h}_{ti}")
                              if mode == "arbitrary":
                                  dcl = p2w.tile([128, 1], F32, tag="dcl",
                                                 name=f"dcl{h}_{ti}")
                                  nc.vector.tensor_scalar_max(
                                      dcl[:], avs[:, H:H + 1], 1e-30)
                                  nc.vector.reciprocal(rec[:], dcl[:])
                              else:
                                  nc.vector.reciprocal(rec[:], avs[:, H:H + 1])
                              tmp = p2w.tile([128, H], BF, tag="avt",
                                             name=f"avt{h}_{ti}")
                              nc.vector.tensor_scalar_mul(tmp[:], avs[:, 0:H],
                                                          rec[:])
                              qk = p2w.tile([128, H], BF, tag="qkg",
                                            name=f"qkg{h}_{ti}")
                              nc.vector.tensor_mul(qk[:], tmp[:], gate[h][ti][:])
                              for c2 in range(2):
                                  tp = psum.tile([128, 128], BF, tag="tp", bufs=2,
                                                 name=f"tp2{h}_{ti}_{c2}")
                                  nc.tensor.transpose(
                                      tp[:], qk[:, c2 * 128:(c2 + 1) * 128],
                                      ident[:])
                                  nc.any.tensor_copy(
                                      qkvgT[2 * h + c2][:, ti * 128:(ti + 1) * 128],
                                      tp[:])
                    # ---- phase 3 for this token block (overlaps next j) ----
                    if 3 in phases:
                      for ti in range(4 * j, 4 * (j + 1)):
                          for db in range(4):
                              po = psum.tile([128, 512], F32, tag="mm", bufs=2,
                                             name=f"po{ti}_{db}")
                              for c in range(2 * HEADS_PC):
                                  nc.tensor.matmul(
                                      po[:], qkvgT[c][:, ti * 128:(ti + 1) * 128],
                                      wo_sb[c][:, db * 512:(db + 1) * 512],
                                      start=(c == 0), stop=(c == 2 * HEADS_PC - 1))
                              ot = p2w.tile([128, 512], BF, tag="ot",
                                            name=f"ot{ti}_{db}")
                              nc.any.tensor_copy(ot[:], po[:])
                              nc.sync.dma_start(
                                  out[ti * 128:(ti + 1) * 128,
                                      db * 512:(db + 1) * 512], ot[:])
                  if 3 not in phases:
                      for c in range(2 * HEADS_PC):
                          dt_ = p2w.tile([128, T], BF, tag="dbg3", name=f"dbg3{c}")
                          nc.vector.tensor_copy(dt_[:], qkvgT[c][:])
                          nc.sync.dma_start(out[c * 128:(c + 1) * 128, :], dt_[:])

    nc.compile()
    return nc


def _get_compiled(mode, apply_w):
    key = (mode, apply_w)
    if key not in _COMPILED:
        _COMPILED[key] = _build(mode, apply_w)
    return _COMPILED[key]


def _rope_tables(positions):
    """Host: exact reference mRoPE sin/cos tables -> CC=[cos|cos], SS=[-sin|sin]."""
    fraction = 2.0 * np.arange(FREQ, dtype=np.float32) / ROTARY
    timescale = (ROPE_THETA ** fraction).astype(np.float32)
    CC, SS = [], []
    for b in range(positions.shape[1]):
        sinusoid = positions[:, b, :, None].astype(np.float32) / timescale
        freq = sinusoid[0].copy()
        h_idx = np.arange(1, 11 * 3, 3)
        w_idx = np.arange(2, 10 * 3, 3)
        freq[:, h_idx] = sinusoid[1][:, h_idx]
        freq[:, w_idx] = sinusoid[2][:, w_idx]
        sin, cos = np.sin(freq), np.cos(freq)
        CC.append(np.concatenate([cos, cos], axis=1).astype(np.float32))
        SS.append(np.concatenate([-sin, sin], axis=1).astype(np.float32))
    return CC, SS


def kernel(x, positions, attn_mask, wq, wk, wv, wo, q_norm_w, k_norm_w):
    global LAST_RESULT, LAST_IN_MAPS
    x = np.asarray(x)
    positions = np.asarray(positions)
    attn_mask = np.asarray(attn_mask)
    wq, wk, wv, wo = map(np.asarray, (wq, wk, wv, wo))
    q_norm_w, k_norm_w = np.asarray(q_norm_w), np.asarray(k_norm_w)

    tril = np.tril(np.ones((T, T), dtype=bool))
    if all(np.array_equal(attn_mask[b], tril) for b in range(B)):
        mode = "causal"
    elif attn_mask.all():
        mode = "full"
    else:
        mode = "arbitrary"
    apply_w = bool(np.any(q_norm_w != 0) or np.any(k_norm_w != 0))

    nc = _get_compiled(mode, apply_w)
    CC, SS = _rope_tables(positions)
    group = N_HEADS // N_KV  # q heads per kv head = 8

    in_maps = []
    for c in range(8):
        b, g = c // 4, c % 4
        kvh = (g * HEADS_PC) // group
        m = {
            "xT": np.ascontiguousarray(x[b].T).astype(BF16),
            "wq": np.ascontiguousarray(
                wq[:, g * HEADS_PC:(g + 1) * HEADS_PC, :]).reshape(
                    D, HEADS_PC * 2 * H).astype(BF16),
            "wkv": np.ascontiguousarray(np.concatenate(
                [wk[:, kvh, :], wv[:, kvh, :]], axis=1)).astype(BF16),
            "wo": np.ascontiguousarray(
                wo[g * HEADS_PC:(g + 1) * HEADS_PC]).reshape(
                    HEADS_PC * H, D).astype(BF16),
            "cc": CC[b].astype(BF16),
            "ss": SS[b].astype(BF16),
        }
        if apply_w:
            m["qw"] = np.ascontiguousarray(np.broadcast_to(
                (1.0 + q_norm_w).astype(np.float32), (128, H)))
            m["kw"] = np.ascontiguousarray(np.broadcast_to(
                (1.0 + k_norm_w).astype(np.float32), (128, H)))
        if mode == "arbitrary":
            m["am"] = np.where(attn_mask[b], np.float32(0.0),
                               np.float32(K_MASK)).astype(np.float32)
        in_maps.append(m)

    res = bass_utils.run_bass_kernel_spmd(nc, in_maps, core_ids=list(range(8)))
    LAST_RESULT = res
    LAST_IN_MAPS = in_maps
    out = np.zeros((B, T, D), np.float32)
    for c in range(8):
        out[c // 4] += res.results[c]["out"].astype(np.float32)
    return out



# revision 3
# speedup vs baseline: 1.0268x; 1.0268x over previous
"""Trainium2 Bass kernel for GQA attention with QK-RMSNorm, partial mRoPE,
causal mask and sigmoid output gate (nn_Attention_70557722739259).

Model dims: B=2, T=2048, D=2048, N=16 Q heads, K=2 KV heads, H=256.
Sharding over 8 NeuronCores: DP=2 over batch x TP=4 over head groups
(4 Q heads + their shared KV head per core). Each core computes a partial
output projection over its 4 heads; the host sums the 4 partials per batch
(Megatron-style unshard).

Per-core device algorithm (all matmuls bf16, softmax f32):
  phase 1 (two half-T passes): projections q/gate/k/v from host-pre-transposed
           x^T, RMSNorm stats via DVE tensor_tensor_reduce, sigmoid gate,
           batched Sqrt for rms, RoPE + norm scale token-major, PE-transpose
           Q,K to feature-major (H, T).
  phase 2: per head, S^T = K^T.T @ Q^T tiles (keys on partitions), exp via
           ACT (1/16 scale folded in; no max subtraction - scores are O(10)),
           multiplicative 0/1 causal mask on diagonal blocks, AV + softmax
           denominator accumulated in PSUM via ones-column in V.
  phase 3: o-proj partial from gated qkv^T (feature-major via PE transpose).
"""
import sys
sys.path.insert(0, "/opt/trn_rl_repo")
import numpy as np
import ml_dtypes

from concourse import bacc, tile, mybir
from concourse import bass_utils
from concourse.masks import make_identity

BF16 = ml_dtypes.bfloat16
F32 = mybir.dt.float32
BF = mybir.dt.bfloat16

B, T, D = 2, 2048, 2048
N_HEADS, N_KV, H = 16, 2, 256
HEADS_PC = 4            # q heads per core (TP=4)
ROPE_THETA = 1000000
ROTARY = 64             # int(H * 0.25)
FREQ = 32
NORM_EPS = 1e-6
K_MASK = -2.3819763e38
SCALE = H ** (-0.5)     # 1/16

TT = T // 128           # 16 token tiles
DC = D // 128           # 16 contraction chunks

NJB = T // 512          # 4 jblocks of 512 tokens

LAST_RESULT = None
LAST_IN_MAPS = None
_COMPILED = {}


def _build_v2():
    """Fast path: causal mask, zero norm weights. Fully feature-major, zero PE
    transposes; see kernel_v2 design notes. Per-head feature permutation PERM
    (host-applied to wq/wk columns) puts RoPE pairs (f, f+32) in the same
    partition of the two half-H tiles, so RoPE is aligned DVE math. RMSNorm
    across partitions via ones-column matmul + gpsimd partition_broadcast.
    Attention: S^T per 128-key chunk; AV^T feature-major with V chunks
    stationary; softmax denominator via ones-column matmul; gate projection
    computed just-in-time in the epilogue to keep the PE busy."""
    nc = bacc.Bacc("TRN2", target_bir_lowering=False, debug=False,
                   enable_asserts=True, num_devices=8)
    Act = mybir.ActivationFunctionType
    Alu = mybir.AluOpType

    xT = nc.dram_tensor("xT", (D, T), BF, kind="ExternalInput").ap()
    wqq = nc.dram_tensor("wqq", (D, HEADS_PC * H), BF, kind="ExternalInput").ap()
    wqg = nc.dram_tensor("wqg", (D, HEADS_PC * H), BF, kind="ExternalInput").ap()
    wk = nc.dram_tensor("wk", (D, H), BF, kind="ExternalInput").ap()
    wv = nc.dram_tensor("wv", (D, H), BF, kind="ExternalInput").ap()
    wo = nc.dram_tensor("wo", (HEADS_PC * H, D), BF, kind="ExternalInput").ap()
    ccf = nc.dram_tensor("ccf", (FREQ, T), BF, kind="ExternalInput").ap()
    ssf = nc.dram_tensor("ssf", (FREQ, T), BF, kind="ExternalInput").ap()
    out = nc.dram_tensor("out", (T, D), BF, kind="ExternalOutput").ap()

    with tile.TileContext(nc) as tc:
        with tc.tile_pool(name="const", bufs=1) as constp, \
             tc.tile_pool(name="persist", bufs=1) as pp, \
             tc.tile_pool(name="psum", bufs=1, space="PSUM") as psum:

            epst = constp.tile([128, 1], F32, tag="epst", name="epst")
            nc.gpsimd.memset(epst[:], NORM_EPS)
            ones_col = constp.tile([128, 1], BF, tag="ones", name="ones")
            nc.gpsimd.memset(ones_col[:], 1.0)
            tri = []
            for r in range(4):
                t = constp.tile([128, 512], BF, tag=f"tri{r}", name=f"tri{r}")
                nc.gpsimd.memset(t[:], 1.0)
                nc.gpsimd.affine_select(
                    out=t[:], in_=t[:], compare_op=Alu.is_ge, fill=0.0,
                    base=-128 * r, channel_multiplier=-1, pattern=[[1, 512]])
                tri.append(t)
            cc_sb = constp.tile([FREQ, T], BF, tag="ccf", name="ccf")
            ss_sb = constp.tile([FREQ, T], BF, tag="ssf", name="ssf")
            nc.sync.dma_start(cc_sb[:], ccf[:])
            nc.sync.dma_start(ss_sb[:], ssf[:])

            xt = [pp.tile([128, T], BF, tag=f"xt{d}", name=f"xt{d}")
                  for d in range(DC)]
            for d in range(DC):
                nc.sync.dma_start(xt[d][:], xT[d * 128:(d + 1) * 128, :])
            QT = [[pp.tile([128, T], BF, tag=f"QT{h}_{hb}", name=f"QT{h}_{hb}")
                   for hb in range(2)] for h in range(HEADS_PC)]
            KT = [pp.tile([128, T], BF, tag=f"KT{hb}", name=f"KT{hb}")
                  for hb in range(2)]
            V = [pp.tile([128, H], BF, tag=f"V{i}", name=f"V{i}")
                 for i in range(TT)]
            qkvgT = [pp.tile([128, T], BF, tag=f"qk{c}", name=f"qk{c}")
                     for c in range(2 * HEADS_PC)]

            # ================= phase A: Q^T, K^T, V =================
            with tc.tile_pool(name="pa", bufs=1) as pa:
                def load_stage_weights(src, suffix):
                    tiles = []
                    for d in range(DC):
                        w_ = pa.tile([128, H], BF, tag=f"wst{d}", bufs=2,
                                     name=f"w{suffix}_{d}")
                        nc.sync.dma_start(
                            w_[:], src[d * 128:(d + 1) * 128, :])
                        tiles.append(w_)
                    return tiles

                pend_ssq = []
                pend_rope = []

                def flush_ssq():
                    while pend_ssq:
                        sq_t, row_t, hb_ = pend_ssq.pop(0)
                        nc.tensor.matmul(row_t[:], ones_col[:], sq_t[:],
                                         start=(hb_ == 0), stop=(hb_ == 1))

                def do_rope_norm(dsts, row_t, jb, nm):
                    c0, c1 = jb * 512, (jb + 1) * 512
                    rms_r = pa.tile([1, 512], F32, tag="rmsr", bufs=2,
                                    name=f"rms{nm}")
                    nc.scalar.activation(rms_r[:], row_t[:], Act.Sqrt,
                                         scale=1.0 / H, bias=epst[0:1])
                    rinv_r = pa.tile([1, 512], F32, tag="rinvr", bufs=2,
                                     name=f"rinv{nm}")
                    nc.vector.reciprocal(rinv_r[:], rms_r[:])
                    rep = pa.tile([128, 512], F32, tag="rep", bufs=2,
                                  name=f"rep{nm}")
                    nc.gpsimd.partition_broadcast(rep[:], rinv_r[:])
                    a = dsts[0][0:FREQ, c0:c1]
                    b = dsts[1][0:FREQ, c0:c1]
                    cs = cc_sb[:, c0:c1]
                    sn = ss_sb[:, c0:c1]
                    t1 = pa.tile([FREQ, 512], BF, tag="r1", bufs=2,
                                 name=f"r1{nm}")
                    t2 = pa.tile([FREQ, 512], BF, tag="r2", bufs=2,
                                 name=f"r2{nm}")
                    t3 = pa.tile([FREQ, 512], BF, tag="r3", bufs=2,
                                 name=f"r3{nm}")
                    nc.vector.tensor_mul(t1[:], a, cs)
                    nc.vector.tensor_mul(t2[:], b, sn)
                    nc.vector.tensor_sub(t1[:], t1[:], t2[:])
                    nc.vector.tensor_mul(t3[:], b, cs)
                    nc.vector.tensor_mul(t2[:], a, sn)
                    nc.vector.tensor_add(t3[:], t3[:], t2[:])
                    nc.vector.tensor_copy(a, t1[:])
                    nc.vector.tensor_copy(b, t3[:])
                    for hb in range(2):
                        sl = dsts[hb][:, c0:c1]
                        nc.vector.tensor_mul(sl, sl, rep[:])

                def flush_rope():
                    while pend_rope:
                        do_rope_norm(*pend_rope.pop(0))

                for stage in range(5):          # h0..h3, K
                    if stage < HEADS_PC:
                        wt = load_stage_weights(
                            wqq[:, stage * H:(stage + 1) * H], f"q{stage}")
                        dsts = QT[stage]
                    else:
                        wt = load_stage_weights(wk, "k")
                        dsts = KT
                    for jb in range(NJB):
                        row_t = psum.tile([1, 512], F32, tag="row", bufs=2,
                                          name=f"row{stage}_{jb}")
                        for hb in range(2):
                            pq = psum.tile([128, 512], F32, tag="mm", bufs=2,
                                           name=f"pq{stage}_{jb}_{hb}")
                            for d in range(DC):
                                nc.tensor.matmul(
                                    pq[:],
                                    wt[d][:, hb * 128:(hb + 1) * 128],
                                    xt[d][:, jb * 512:(jb + 1) * 512],
                                    start=(d == 0), stop=(d == DC - 1))
                            flush_ssq()
                            raw = dsts[hb][:, jb * 512:(jb + 1) * 512]
                            nc.vector.tensor_copy(raw, pq[:])
                            sq = pa.tile([128, 512], BF, tag="sq", bufs=3,
                                         name=f"sq{stage}_{jb}_{hb}")
                            nc.scalar.square(sq[:], pq[:])
                            pend_ssq.append((sq, row_t, hb))
                        pend_rope.append((dsts, row_t, jb, f"{stage}_{jb}"))
                        if len(pend_rope) > 1:
                            do_rope_norm(*pend_rope.pop(0))
                wvt = load_stage_weights(wv, "v")
                for ti in range(TT):
                    pv = psum.tile([128, 512], F32, tag="po", bufs=2,
                                   name=f"pv{ti}")
                    for d in range(DC):
                        nc.tensor.matmul(
                            pv[:, 0:H], xt[d][:, ti * 128:(ti + 1) * 128],
                            wvt[d][:], start=(d == 0), stop=(d == DC - 1))
                    if ti == 0:
                        flush_ssq()
                        flush_rope()
                    nc.vector.tensor_copy(V[ti][:], pv[:, 0:H])

            # ================= phase B: attention + gate =================
            with tc.tile_pool(name="pb", bufs=1) as pb:
                avT = [psum.tile([128, 512], F32, tag=f"avT{hb}", bufs=1,
                                 name=f"avT{hb}") for hb in range(2)]
                for h in range(HEADS_PC):
                    wgt = []
                    for d in range(DC):
                        w_ = pb.tile([128, H], BF, tag=f"wg{d}",
                                     name=f"wg{h}_{d}", bufs=2)
                        nc.sync.dma_start(
                            w_[:], wqg[d * 128:(d + 1) * 128,
                                       h * H:(h + 1) * H])
                        wgt.append(w_)
                    for jb in range(NJB):
                        kmax = 4 * (jb + 1)
                        den = psum.tile([1, 512], F32, tag="row", bufs=2,
                                        name=f"den{h}_{jb}")
                        c0, c1 = jb * 512, (jb + 1) * 512
                        pend = []

                        def emit_av(pT_, i_):
                            st_, sp_ = (i_ == 0), (i_ == kmax - 1)
                            for hb in range(2):
                                nc.tensor.matmul(
                                    avT[hb][:],
                                    V[i_][:, hb * 128:(hb + 1) * 128],
                                    pT_[:], start=st_, stop=sp_)
                            nc.tensor.matmul(den[:], ones_col[:], pT_[:],
                                             start=st_, stop=sp_)

                        for i in range(kmax):
                            st = psum.tile([128, 512], F32, tag="mm", bufs=2,
                                           name=f"st{h}_{jb}_{i}")
                            for hb in range(2):
                                nc.tensor.matmul(
                                    st[:], KT[hb][:, i * 128:(i + 1) * 128],
                                    QT[h][hb][:, c0:c1],
                                    start=(hb == 0), stop=(hb == 1))
                            pT = pb.tile([128, 512], BF, tag="pT", bufs=3,
                                         name=f"pT{h}_{jb}_{i}")
                            nc.scalar.activation(pT[:], st[:], Act.Exp,
                                                 scale=SCALE)
                            if i >= 4 * jb:
                                nc.vector.tensor_mul(pT[:], pT[:],
                                                     tri[i - 4 * jb][:])
                            pend.append((pT, i))
                            if len(pend) > 1:
                                emit_av(*pend.pop(0))
                        emit_av(*pend.pop(0))

                        gt = []
                        for hb in range(2):
                            pg = psum.tile([128, 512], F32, tag="po", bufs=2,
                                           name=f"pg{h}_{jb}_{hb}")
                            for d in range(DC):
                                nc.tensor.matmul(
                                    pg[:],
                                    wgt[d][:, hb * 128:(hb + 1) * 128],
                                    xt[d][:, c0:c1],
                                    start=(d == 0), stop=(d == DC - 1))
                            g_ = pb.tile([128, 512], BF, tag="gt", bufs=2,
                                         name=f"gt{h}_{jb}_{hb}")
                            nc.scalar.activation(g_[:], pg[:], Act.Sigmoid)
                            gt.append(g_)

                        den_s = pb.tile([1, 512], F32, tag="dens", bufs=2,
                                        name=f"dens{h}_{jb}")
                        nc.vector.tensor_copy(den_s[:], den[:])
                        rec = pb.tile([1, 512], F32, tag="rec", bufs=2,
                                      name=f"rec{h}_{jb}")
                        nc.vector.reciprocal(rec[:], den_s[:])
                        dpr = pb.tile([128, 512], F32, tag="dpr", bufs=2,
                                      name=f"dpr{h}_{jb}")
                        nc.gpsimd.partition_broadcast(dpr[:], rec[:])
                        for hb in range(2):
                            avs = pb.tile([128, 512], BF, tag="avs", bufs=2,
                                          name=f"avs{h}_{jb}_{hb}")
                            nc.scalar.copy(avs[:], avT[hb][:])
                            nc.vector.tensor_mul(avs[:], avs[:], dpr[:])
                            nc.vector.tensor_mul(
                                qkvgT[2 * h + hb][:, c0:c1], avs[:], gt[hb][:])

            # ================= phase C: o-proj =================
            with tc.tile_pool(name="pc", bufs=1) as pc_:
                for db in range(4):
                    wod = []
                    for c in range(2 * HEADS_PC):
                        w_ = pc_.tile([128, 512], BF, tag=f"wod{c}",
                                      name=f"wod{db}_{c}", bufs=2)
                        nc.sync.dma_start(
                            w_[:], wo[c * 128:(c + 1) * 128,
                                      db * 512:(db + 1) * 512])
                        wod.append(w_)
                    for ti in range(TT):
                        po = psum.tile([128, 512], F32, tag="mm", bufs=2,
                                       name=f"po{db}_{ti}")
                        for c in range(2 * HEADS_PC):
                            nc.tensor.matmul(
                                po[:], qkvgT[c][:, ti * 128:(ti + 1) * 128],
                                wod[c][:], start=(c == 0),
                                stop=(c == 2 * HEADS_PC - 1))
                        ot = pc_.tile([128, 512], BF, tag="ot", bufs=3,
                                      name=f"ot{db}_{ti}")
                        nc.scalar.copy(ot[:], po[:])
                        nc.sync.dma_start(
                            out[ti * 128:(ti + 1) * 128,
                                db * 512:(db + 1) * 512], ot[:])

    nc.compile()
    return nc


# feature permutation: tileA rows = feats [0:32)+[64:160), tileB = [32:64)+[160:256)
PERM = np.concatenate([np.arange(0, 32), np.arange(64, 160),
                       np.arange(32, 64), np.arange(160, 256)])


def _rope_tables_fm(positions):
    """Feature-major mRoPE tables per batch: cosF, sinF [32, T]."""
    fraction = 2.0 * np.arange(FREQ, dtype=np.float32) / ROTARY
    timescale = (ROPE_THETA ** fraction).astype(np.float32)
    CCF, SSF = [], []
    for b in range(positions.shape[1]):
        sinusoid = positions[:, b, :, None].astype(np.float32) / timescale
        freq = sinusoid[0].copy()
        h_idx = np.arange(1, 11 * 3, 3)
        w_idx = np.arange(2, 10 * 3, 3)
        freq[:, h_idx] = sinusoid[1][:, h_idx]
        freq[:, w_idx] = sinusoid[2][:, w_idx]
        CCF.append(np.ascontiguousarray(np.cos(freq).T).astype(BF16))
        SSF.append(np.ascontiguousarray(np.sin(freq).T).astype(BF16))
    return CCF, SSF


def _make_in_maps_v2(x, positions, wq, wk, wv, wo):
    CCF, SSF = _rope_tables_fm(positions)
    group = N_HEADS // N_KV
    in_maps = []
    for c in range(8):
        b, g = c // 4, c % 4
        kvh = (g * HEADS_PC) // group
        heads = slice(g * HEADS_PC, (g + 1) * HEADS_PC)
        wq_h = wq[:, heads, :]
        wqq = wq_h[:, :, 0:H][:, :, PERM]
        wqg = wq_h[:, :, H:2 * H]
        m = {
            "xT": np.ascontiguousarray(x[b].T).astype(BF16),
            "wqq": np.ascontiguousarray(
                wqq.reshape(D, HEADS_PC * H)).astype(BF16),
            "wqg": np.ascontiguousarray(
                wqg.reshape(D, HEADS_PC * H)).astype(BF16),
            "wk": np.ascontiguousarray(wk[:, kvh, :][:, PERM]).astype(BF16),
            "wv": np.ascontiguousarray(wv[:, kvh, :]).astype(BF16),
            "wo": np.ascontiguousarray(
                wo[heads].reshape(HEADS_PC * H, D)).astype(BF16),
            "ccf": CCF[b],
            "ssf": SSF[b],
        }
        in_maps.append(m)
    return in_maps


def _build(mode="causal", apply_w=False, phases=(1, 2, 3), n_halves=2, n_heads_dbg=HEADS_PC):
    nc = bacc.Bacc("TRN2", target_bir_lowering=False, debug=False,
                   enable_asserts=True, num_devices=8)
    Act = mybir.ActivationFunctionType
    Alu = mybir.AluOpType

    xT = nc.dram_tensor("xT", (D, T), BF, kind="ExternalInput").ap()
    wq = nc.dram_tensor("wq", (D, HEADS_PC * 2 * H), BF, kind="ExternalInput").ap()
    wkv = nc.dram_tensor("wkv", (D, 2 * H), BF, kind="ExternalInput").ap()
    wo = nc.dram_tensor("wo", (HEADS_PC * H, D), BF, kind="ExternalInput").ap()
    cc = nc.dram_tensor("cc", (T, ROTARY), BF, kind="ExternalInput").ap()
    ss = nc.dram_tensor("ss", (T, ROTARY), BF, kind="ExternalInput").ap()
    if apply_w:
        qw = nc.dram_tensor("qw", (128, H), F32, kind="ExternalInput").ap()
        kw = nc.dram_tensor("kw", (128, H), F32, kind="ExternalInput").ap()
    if mode == "arbitrary":
        am = nc.dram_tensor("am", (T, T), F32, kind="ExternalInput").ap()
    out = nc.dram_tensor("out", (T, D), BF, kind="ExternalOutput").ap()

    with tile.TileContext(nc) as tc:
        with tc.tile_pool(name="const", bufs=1) as constp, \
             tc.tile_pool(name="attn", bufs=1) as ap_, \
             tc.tile_pool(name="psum", bufs=1, space="PSUM") as psum:

            # ---- constants ----
            ident = constp.tile([128, 128], BF, tag="ident", name="ident")
            make_identity(nc, ident[:])
            epst = constp.tile([128, 1], F32, tag="epst", name="epst")
            nc.gpsimd.memset(epst[:], NORM_EPS)
            tri = []
            if mode == "causal":
                for r in range(4):
                    t = constp.tile([128, 512], BF, tag=f"tri{r}", name=f"tri{r}")
                    nc.gpsimd.memset(t[:], 1.0)
                    # keep (1.0) where col - part - 128*r >= 0, else fill 0.0
                    nc.gpsimd.affine_select(
                        out=t[:], in_=t[:], compare_op=Alu.is_ge, fill=0.0,
                        base=-128 * r, channel_multiplier=-1, pattern=[[1, 512]])
                    tri.append(t)
            if apply_w:
                qw_sb = constp.tile([128, H], F32, tag="qw", name="qw")
                kw_sb = constp.tile([128, H], F32, tag="kw", name="kw")
                nc.sync.dma_start(qw_sb[:], qw[:])
                nc.sync.dma_start(kw_sb[:], kw[:])

            # ---- persistent attention tensors (span phase 1 -> 2) ----
            QT = [[ap_.tile([128, T], BF, tag=f"QT{h}_{c}", name=f"QT{h}_{c}")
                   for c in range(2)] for h in range(HEADS_PC)]
            KT = [ap_.tile([128, T], BF, tag=f"KT{c}", name=f"KT{c}")
                  for c in range(2)]
            V = [ap_.tile([128, H + 1], BF, tag=f"V{i}", name=f"V{i}")
                 for i in range(TT)]
            gate = [[ap_.tile([128, H], BF, tag=f"g{h}_{i}", name=f"g{h}_{i}")
                     for i in range(TT)] for h in range(HEADS_PC)]

            # ================= phase 1 (two half-T passes) =================
            with tc.tile_pool(name="praw", bufs=1) as praw, \
                 tc.tile_pool(name="proj", bufs=1) as proj, \
                 tc.tile_pool(name="p1w", bufs=3) as p1w, \
                 tc.tile_pool(name="p1c", bufs=3) as p1c:
                wkv_sb = [proj.tile([128, 2 * H], BF, tag=f"wkv{d}",
                                    name=f"wkv{d}") for d in range(DC)]
                for d in range(DC):
                    nc.sync.dma_start(wkv_sb[d][:],
                                      wkv[d * 128:(d + 1) * 128, :])

                n_half = HEADS_PC * 8 + 8   # ssq columns per half (q then k)

                for half in range(n_halves):
                    t0 = half * 1024
                    xT_sb = []
                    for d in range(DC):
                        xt_ = proj.tile([128, 1024], BF, tag=f"xT{d}",
                                        name=f"xT{d}_{half}")
                        nc.sync.dma_start(
                            xt_[:], xT[d * 128:(d + 1) * 128, t0:t0 + 1024])
                        xT_sb.append(xt_)
                    cc_sb, ss_sb = [], []
                    for tl in range(8):
                        ti = half * 8 + tl
                        c_ = praw.tile([128, ROTARY], BF, tag=f"cc{tl}",
                                       name=f"cc{tl}_{half}")
                        s_ = praw.tile([128, ROTARY], BF, tag=f"ss{tl}",
                                       name=f"ss{tl}_{half}")
                        nc.sync.dma_start(c_[:], cc[ti * 128:(ti + 1) * 128, :])
                        nc.sync.dma_start(s_[:], ss[ti * 128:(ti + 1) * 128, :])
                        cc_sb.append(c_)
                        ss_sb.append(s_)

                    ssqall = praw.tile([128, n_half], F32, tag="ssq", bufs=2,
                                       name=f"ssq{half}")
                    q_raw = [[praw.tile([128, H], BF, tag=f"qr{h}_{tl}",
                                        name=f"qr{h}_{tl}_{half}")
                              for tl in range(8)] for h in range(HEADS_PC)]
                    k_raw = [praw.tile([128, H], BF, tag=f"kr{tl}",
                                       name=f"kr{tl}_{half}")
                             for tl in range(8)]

                    # ---- pass A: projections ----
                    for tl in range(8):
                        ti = half * 8 + tl
                        pk = psum.tile([128, 2 * H], F32, tag="mm", bufs=2,
                                       name=f"pk{ti}")
                        for d in range(DC):
                            nc.tensor.matmul(
                                pk[:], xT_sb[d][:, tl * 128:(tl + 1) * 128],
                                wkv_sb[d][:], start=(d == 0), stop=(d == DC - 1))
                        kvt = p1w.tile([128, 2 * H], BF, tag="fulltmp",
                                       name=f"kvt{ti}")
                        nc.scalar.copy(kvt[:], pk[:])
                        nc.vector.tensor_copy(k_raw[tl][:], kvt[:, 0:H])
                        nc.vector.tensor_copy(V[ti][:, 0:H], kvt[:, H:2 * H])
                        nc.gpsimd.memset(V[ti][:, H:H + 1], 1.0)
                        junk = p1w.tile([128, H], F32, tag="junk", name=f"jk{ti}")
                        nc.scalar.square(junk[:], k_raw[tl][:])
                        nc.vector.reduce_sum(ssqall[:, 32 + tl:32 + tl + 1],
                                             junk[:], axis=mybir.AxisListType.X)

                    for h in range(n_heads_dbg):
                        wq_h = []
                        for d in range(DC):
                            w_ = proj.tile([128, 512], BF, tag=f"wq{d}",
                                           name=f"wq{d}_{half}_{h}")
                            nc.sync.dma_start(
                                w_[:], wq[d * 128:(d + 1) * 128,
                                          h * 512:(h + 1) * 512])
                            wq_h.append(w_)
                        for tl in range(8):
                            ti = half * 8 + tl
                            pq = psum.tile([128, 2 * H], F32, tag="mm", bufs=2,
                                           name=f"pq{h}_{ti}")
                            for d in range(DC):
                                nc.tensor.matmul(
                                    pq[:], xT_sb[d][:, tl * 128:(tl + 1) * 128],
                                    wq_h[d][:], start=(d == 0),
                                    stop=(d == DC - 1))
                            qgt = p1w.tile([128, 2 * H], BF, tag="fulltmp",
                                           name=f"qgt{h}_{ti}")
                            nc.scalar.copy(qgt[:], pq[:])
                            nc.vector.tensor_copy(q_raw[h][tl][:], qgt[:, 0:H])
                            nc.scalar.activation(gate[h][ti][:], qgt[:, H:2 * H],
                                                 Act.Sigmoid)
                            junk = p1w.tile([128, H], F32, tag="junk",
                                            name=f"jq{h}_{ti}")
                            nc.scalar.square(junk[:], q_raw[h][tl][:])
                            nc.vector.reduce_sum(
                                ssqall[:, h * 8 + tl:h * 8 + tl + 1],
                                junk[:], axis=mybir.AxisListType.X)

                    # ---- pass B: batched rms scales for this half ----
                    rms = praw.tile([128, n_half], F32, tag="rms", bufs=2,
                                    name=f"rms{half}")
                    rinv = praw.tile([128, n_half], F32, tag="rinv", bufs=2,
                                     name=f"rinv{half}")
                    nc.scalar.activation(rms[:], ssqall[:], Act.Sqrt,
                                         scale=1.0 / H, bias=epst[:])
                    nc.vector.reciprocal(rinv[:], rms[:])

                    # ---- pass C: rope + norm scale + transpose ----
                    def rope_pass(raw, col, dst_tiles, tl, w_sb, nm):
                        ti = half * 8 + tl
                        src = raw
                        if apply_w:
                            srw = p1c.tile([128, H], F32, tag="srw",
                                           name=f"srw{nm}")
                            nc.vector.tensor_mul(srw[:], raw[:], w_sb[:])
                            src = srw
                        rot = p1c.tile([128, ROTARY], BF, tag="rot",
                                       name=f"rot{nm}")
                        t2 = p1c.tile([128, ROTARY], BF, tag="rot2",
                                      name=f"rot2{nm}")
                        nc.vector.tensor_mul(rot[:], src[:, 0:ROTARY],
                                             cc_sb[tl][:])
                        nc.vector.tensor_mul(t2[:, 0:FREQ], src[:, FREQ:ROTARY],
                                             ss_sb[tl][:, 0:FREQ])
                        nc.vector.tensor_mul(t2[:, FREQ:ROTARY], src[:, 0:FREQ],
                                             ss_sb[tl][:, FREQ:ROTARY])
                        nc.vector.tensor_add(rot[:], rot[:], t2[:])
                        tok = p1c.tile([128, H], BF, tag="tok", name=f"tok{nm}")
                        rv = rinv[:, col:col + 1]
                        nc.vector.tensor_scalar_mul(tok[:, 0:ROTARY], rot[:], rv)
                        nc.vector.tensor_scalar_mul(tok[:, ROTARY:H],
                                                    src[:, ROTARY:H], rv)
                        for c2 in range(2):
                            tp = psum.tile([128, 128], BF, tag="tp", bufs=2,
                                           name=f"tp{nm}_{c2}")
                            nc.tensor.transpose(
                                tp[:], tok[:, c2 * 128:(c2 + 1) * 128], ident[:])
                            nc.any.tensor_copy(
                                dst_tiles[c2][:, ti * 128:(ti + 1) * 128], tp[:])

                    for tl in range(8):
                        rope_pass(k_raw[tl], 32 + tl, KT, tl,
                                  kw_sb if apply_w else None, f"k{half}_{tl}")
                    for h in range(n_heads_dbg):
                        for tl in range(8):
                            rope_pass(q_raw[h][tl], h * 8 + tl, QT[h], tl,
                                      qw_sb if apply_w else None,
                                      f"q{h}_{half}_{tl}")

            # ================= phases 2+3 =================
            if phases == (1,):
                # debug: dump K^T and QT[0] so HW output is defined
                with tc.tile_pool(name="dbg", bufs=2) as dbg:
                    for idx, tile_ in enumerate([KT[0], KT[1], QT[0][0],
                                                 QT[0][1], QT[1][0], QT[1][1]]):
                        dt_ = dbg.tile([128, T], BF, tag="dbg", name=f"dbg{idx}")
                        nc.vector.tensor_copy(dt_[:], tile_[:])
                        nc.sync.dma_start(out[idx * 128:(idx + 1) * 128, :], dt_[:])
                    for idx2 in range(6, 16):
                        ti2 = idx2 - 6
                        dt_ = dbg.tile([128, T], BF, tag="dbg", name=f"dbg{idx2}")
                        nc.gpsimd.memset(dt_[:], 0.0)
                        nc.vector.tensor_copy(dt_[:, 0:H], V[ti2][:, 0:H])
                        nc.vector.tensor_copy(dt_[:, H:2 * H], gate[0][ti2][:])
                        nc.sync.dma_start(out[idx2 * 128:(idx2 + 1) * 128, :],
                                          dt_[:])
            if phases != (1,):
              with tc.tile_pool(name="p23", bufs=1) as p23, \
                 tc.tile_pool(name="p2w", bufs=3) as p2w:
                  qkvgT = [p23.tile([128, T], BF, tag=f"qkT{c}", name=f"qkT{c}")
                           for c in range(2 * HEADS_PC)]
                  wo_sb = [p23.tile([128, D], BF, tag=f"wo{c}", name=f"wo{c}")
                           for c in range(2 * HEADS_PC)]
                  for c in range(2 * HEADS_PC):
                      nc.sync.dma_start(wo_sb[c][:], wo[c * 128:(c + 1) * 128, :])

                  for j in range(4):              # q blocks of 512
                    kmax = 4 * (j + 1) if mode == "causal" else TT
                    for h in range(HEADS_PC):
                          av = [psum.tile([128, H + 1], F32, tag=f"av{s}", bufs=1,
                                          name=f"av{h}_{j}_{s}") for s in range(4)]
                          for i in range(kmax):   # key chunks of 128
                              st = psum.tile([128, 512], F32, tag="mm", bufs=2,
                                             name=f"st{h}_{j}_{i}")
                              for c2 in range(2):
                                  nc.tensor.matmul(
                                      st[:],
                                      KT[c2][:, i * 128:(i + 1) * 128],
                                      QT[h][c2][:, j * 512:(j + 1) * 512],
                                      start=(c2 == 0), stop=(c2 == 1))
                              if mode == "arbitrary":
                                  amt = p2w.tile([128, 512], F32, tag="amt",
                                                 name=f"am{h}_{j}_{i}")
                                  nc.sync.dma_start(
                                      amt[:], am[i * 128:(i + 1) * 128,
                                                 j * 512:(j + 1) * 512])
                                  nc.vector.tensor_add(st[:], st[:], amt[:])
                              pT = p2w.tile([128, 512], BF, tag="pT",
                                            name=f"pT{h}_{j}_{i}")
                              nc.scalar.activation(pT[:], st[:], Act.Exp,
                                                   scale=SCALE)
                              if mode == "causal" and i >= 4 * j:
                                  nc.vector.tensor_mul(pT[:], pT[:],
                                                       tri[i - 4 * j][:])
                              for s in range(4):
                                  last_i = (4 * j + s) if mode == "causal" \
                                      else (kmax - 1)
                                  if i > last_i:
                                      continue
                                  nc.tensor.matmul(
                                      av[s][:], pT[:, s * 128:(s + 1) * 128],
                                      V[i][:], start=(i == 0),
                                      stop=(i == last_i))
                          for s in range(4):
                              ti = 4 * j + s
                              avs = p2w.tile([128, H + 1], F32, tag="avs",
                                             name=f"avs{h}_{ti}")
                              nc.vector.tensor_copy(avs[:], av[s][:])
                              rec = p2w.tile([128, 1], F32, tag="rec",
                                             name=f"rec{h}_{ti}")
                              if mode == "arbitrary":
                                  dcl = p2w.tile([128, 1], F32, tag="dcl",
                                                 name=f"dcl{h}_{ti}")
                                  nc.vector.tensor_scalar_max(
                                      dcl[:], avs[:, H:H + 1], 1e-30)
                                  nc.vector.reciprocal(rec[:], dcl[:])
                              else:
                                  nc.vector.reciprocal(rec[:], avs[:, H:H + 1])
                              tmp = p2w.tile([128, H], BF, tag="avt",
                                             name=f"avt{h}_{ti}")
                              nc.vector.tensor_scalar_mul(tmp[:], avs[:, 0:H],
                                                          rec[:])
                              qk = p2w.tile([128, H], BF, tag="qkg",
                                            name=f"qkg{h}_{ti}")
                              nc.vector.tensor_mul(qk[:], tmp[:], gate[h][ti][:])
                              for c2 in range(2):
                                  tp = psum.tile([128, 128], BF, tag="tp", bufs=2,
                                                 name=f"tp2{h}_{ti}_{c2}")
                                  nc.tensor.transpose(
                                      tp[:], qk[:, c2 * 128:(c2 + 1) * 128],
                                      ident[:])
                                  nc.any.tensor_copy(
                                      qkvgT[2 * h + c2][:, ti * 128:(ti + 1) * 128],
                                      tp[:])
                    # ---- phase 3 for this token block (overlaps next j) ----
                    if 3 in phases:
                      for ti in range(4 * j, 4 * (j + 1)):
                          for db in range(4):
                              po = psum.tile([128, 512], F32, tag="mm", bufs=2,
                                             name=f"po{ti}_{db}")
                              for c in range(2 * HEADS_PC):
                                  nc.tensor.matmul(
                                      po[:], qkvgT[c][:, ti * 128:(ti + 1) * 128],
                                      wo_sb[c][:, db * 512:(db + 1) * 512],
                                      start=(c == 0), stop=(c == 2 * HEADS_PC - 1))
                              ot = p2w.tile([128, 512], BF, tag="ot",
                                            name=f"ot{ti}_{db}")
                              nc.any.tensor_copy(ot[:], po[:])
                              nc.sync.dma_start(
                                  out[ti * 128:(ti + 1) * 128,
                                      db * 512:(db + 1) * 512], ot[:])
                  if 3 not in phases:
                      for c in range(2 * HEADS_PC):
                          dt_ = p2w.tile([128, T], BF, tag="dbg3", name=f"dbg3{c}")
                          nc.vector.tensor_copy(dt_[:], qkvgT[c][:])
                          nc.sync.dma_start(out[c * 128:(c + 1) * 128, :], dt_[:])

    nc.compile()
    return nc


def _get_compiled(mode, apply_w):
    key = (mode, apply_w)
    if key not in _COMPILED:
        _COMPILED[key] = _build(mode, apply_w)
    return _COMPILED[key]


def _rope_tables(positions):
    """Host: exact reference mRoPE sin/cos tables -> CC=[cos|cos], SS=[-sin|sin]."""
    fraction = 2.0 * np.arange(FREQ, dtype=np.float32) / ROTARY
    timescale = (ROPE_THETA ** fraction).astype(np.float32)
    CC, SS = [], []
    for b in range(positions.shape[1]):
        sinusoid = positions[:, b, :, None].astype(np.float32) / timescale
        freq = sinusoid[0].copy()
        h_idx = np.arange(1, 11 * 3, 3)
        w_idx = np.arange(2, 10 * 3, 3)
        freq[:, h_idx] = sinusoid[1][:, h_idx]
        freq[:, w_idx] = sinusoid[2][:, w_idx]
        sin, cos = np.sin(freq), np.cos(freq)
        CC.append(np.concatenate([cos, cos], axis=1).astype(np.float32))
        SS.append(np.concatenate([-sin, sin], axis=1).astype(np.float32))
    return CC, SS


def kernel(x, positions, attn_mask, wq, wk, wv, wo, q_norm_w, k_norm_w):
    global LAST_RESULT, LAST_IN_MAPS
    x = np.asarray(x)
    positions = np.asarray(positions)
    attn_mask = np.asarray(attn_mask)
    wq, wk, wv, wo = map(np.asarray, (wq, wk, wv, wo))
    q_norm_w, k_norm_w = np.asarray(q_norm_w), np.asarray(k_norm_w)

    tril = np.tril(np.ones((T, T), dtype=bool))
    if all(np.array_equal(attn_mask[b], tril) for b in range(B)):
        mode = "causal"
    elif attn_mask.all():
        mode = "full"
    else:
        mode = "arbitrary"
    apply_w = bool(np.any(q_norm_w != 0) or np.any(k_norm_w != 0))

    if mode == "causal" and not apply_w:
        if "v2" not in _COMPILED:
            _COMPILED["v2"] = _build_v2()
        nc = _COMPILED["v2"]
        in_maps = _make_in_maps_v2(x, positions, wq, wk, wv, wo)
        res = bass_utils.run_bass_kernel_spmd(nc, in_maps,
                                              core_ids=list(range(8)))
        LAST_RESULT = res
        LAST_IN_MAPS = in_maps
        out = np.zeros((B, T, D), np.float32)
        for c in range(8):
            out[c // 4] += res.results[c]["out"].astype(np.float32)
        return out

    nc = _get_compiled(mode, apply_w)
    CC, SS = _rope_tables(positions)
    group = N_HEADS // N_KV  # q heads per kv head = 8

    in_maps = []
    for c in range(8):
        b, g = c // 4, c % 4
        kvh = (g * HEADS_PC) // group
        m = {
            "xT": np.ascontiguousarray(x[b].T).astype(BF16),
            "wq": np.ascontiguousarray(
                wq[:, g * HEADS_PC:(g + 1) * HEADS_PC, :]).reshape(
                    D, HEADS_PC * 2 * H).astype(BF16),
            "wkv": np.ascontiguousarray(np.concatenate(
                [wk[:, kvh, :], wv[:, kvh, :]], axis=1)).astype(BF16),
            "wo": np.ascontiguousarray(
                wo[g * HEADS_PC:(g + 1) * HEADS_PC]).reshape(
                    HEADS_PC * H, D).astype(BF16),
            "cc": CC[b].astype(BF16),
            "ss": SS[b].astype(BF16),
        }
        if apply_w:
            m["qw"] = np.ascontiguousarray(np.broadcast_to(
                (1.0 + q_norm_w).astype(np.float32), (128, H)))
            m["kw"] = np.ascontiguousarray(np.broadcast_to(
                (1.0 + k_norm_w).astype(np.float32), (128, H)))
        if mode == "arbitrary":
            m["am"] = np.where(attn_mask[b], np.float32(0.0),
                               np.float32(K_MASK)).astype(np.float32)
        in_maps.append(m)

    res = bass_utils.run_bass_kernel_spmd(nc, in_maps, core_ids=list(range(8)))
    LAST_RESULT = res
    LAST_IN_MAPS = in_maps
    out = np.zeros((B, T, D), np.float32)
    for c in range(8):
        out[c // 4] += res.results[c]["out"].astype(np.float32)
    return out

